# revision 11
# baseline (speedup 1.0000x reference)
"""Complex attention kernel for Trainium2, 8 NeuronCores.

reference math (B=4, S=4096, H=256, h=128):
    scores  = (Qre@Kre^T + Qim@Kim^T) / 16          [B,S,S]
    weights = softmax(scores, -1)                    [B,S,S]  (output 2)
    out     = concat(weights@Vre, weights@Vim, -1)   [B,S,H]  (output 1)

Sharding: 8 shards = (batch b, half of Q sequence). Each core gets
Q[b, s0:s0+2048], K[b], V[b]; no collectives needed.

Per-core pipeline:
  prologue: load K,Q natural -> PE-transpose to [d,seq] f32r layout (fp32
  DMA transpose is unsupported); V loads natural + DVE cast to f32r.
  stage 2 (natural layout, per 128-row q tile):
      S tile [128,1024] = two accumulating f32r matmuls per 512 cols
      E = exp(S/16) on ACT with accum_out rowsum partials
      recip = 1/rowsum (DVE), W = E*recip (DVE/gpsimd alternating), DMA out.
  stage 3 (transposed layout, per 1024-col q chunk):
      S^T tile [k=128, q=1024] recomputed with swapped matmul operands
      E^T = exp(S^T/16); PV accumulates out^T[d, q] over 32 k tiles with V
      natural as stationary; epilogue PE-transposes out^T back to [q, d],
      DVE-scales by the stage-2 reciprocal, DMAs out.
softmax uses no max-subtraction: scores are O(+-7), exp is safe in f32 and
the result is mathematically identical to the max-subtracted reference.
"""

import numpy as np
from contextlib import ExitStack

import concourse.bass as bass
import concourse.bacc as bacc
import concourse.mybir as mybir
import concourse.tile as tile
from concourse.bass_utils import run_bass_kernel_spmd
from concourse.masks import make_identity

B, S, H = 4, 4096, 256
HD = H // 2                      # 128, half hidden (re/im)
N_CORES = 8
QS = B * S // N_CORES            # 2048 q rows per core
INV_SCALE = 1.0 / 16.0           # 1/sqrt(H//2)

F32 = mybir.dt.float32
F32R = mybir.dt.float32r
EXP = mybir.ActivationFunctionType.Exp
AX = mybir.AxisListType.X

P = 128
NQ = QS // P                     # 16 q tiles
NK = S // P                      # 32 k tiles
KC = 1024                        # stage2 k-chunk width (psum/exp tile)
NKC = S // KC                    # 4
QC = 1024                        # stage3 q-chunk width
NQC = QS // QC                   # 2

LAST_RESULTS = None              # test harness peeks at exec_time_ns


def _body(ctx, tc, Qd, Kd, Vd, OUTd, Wd):
    nc = tc.nc

    singles = ctx.enter_context(tc.tile_pool(name="singles", bufs=1))
    ident = singles.tile([P, P], F32)
    make_identity(nc, ident)

    KtRe = singles.tile([P, S], F32R)
    KtIm = singles.tile([P, S], F32R)
    QtRe = singles.tile([P, QS], F32R)
    QtIm = singles.tile([P, QS], F32R)
    Vsb = singles.tile([P, NK, H], F32R)
    recips = singles.tile([P, NQ], F32)

    # PSUM: tag "ps" 2 slots x 2 banks (S / S^T / all transposes),
    #       tag "o" 2 slots x 2 banks (out^T re+im accumulators) = 8 banks.
    psS = ctx.enter_context(tc.tile_pool(name="psS", bufs=2, space="PSUM"))
    psOut = ctx.enter_context(tc.tile_pool(name="psOut", bufs=2, space="PSUM"))
    nat = ctx.enter_context(tc.tile_pool(name="nat", bufs=4))
    esb = ctx.enter_context(tc.tile_pool(name="esb", bufs=8))
    wsb = ctx.enter_context(tc.tile_pool(name="wsb", bufs=4))
    etp = ctx.enter_context(tc.tile_pool(name="etp", bufs=4))
    outp = ctx.enter_context(tc.tile_pool(name="outp", bufs=4))
    small = ctx.enter_context(tc.tile_pool(name="small", bufs=4))

    # prologue: batched natural loads (8 seq-tiles per DMA), PE transposes
    # with PSUM->SBUF copies split between DVE and ACT. K first (it gates
    # stage2), then Q, then V (cast to f32r on gpsimd).
    GRP = 8

    def load_transpose(src_dram, grp, n_in_grp, dstRe, dstIm):
        nat_t = nat.tile([P, GRP, H], F32, tag="nat")
        src = src_dram.rearrange("(n p) h -> p n h", p=P)
        nc.sync.dma_start(
            out=nat_t[:, :n_in_grp, :],
            in_=src[:, grp * GRP:grp * GRP + n_in_grp, :])
        for j in range(n_in_grp):
            idx = grp * GRP + j
            for h, dst in ((0, dstRe), (1, dstIm)):
                pt = psS.tile([P, P], F32, tag="ps")
                nc.tensor.transpose(pt, nat_t[:, j, h * HD:(h + 1) * HD], ident)
                if (idx + h) % 2 == 0:
                    nc.vector.tensor_copy(
                        out=dst[:, idx * P:(idx + 1) * P], in_=pt)
                else:
                    nc.scalar.copy(out=dst[:, idx * P:(idx + 1) * P], in_=pt)

    for grp in range(NK // GRP):
        load_transpose(Kd, grp, GRP, KtRe, KtIm)
    for grp in range(NQ // GRP):
        load_transpose(Qd, grp, GRP, QtRe, QtIm)
    Vre = Vd.rearrange("(n p) h -> p n h", p=P)
    for grp in range(NK // GRP):
        vnat = nat.tile([P, GRP, H], F32, tag="nat")
        nc.sync.dma_start(out=vnat, in_=Vre[:, grp * GRP:(grp + 1) * GRP, :])
        nc.gpsimd.tensor_copy(
            out=Vsb[:, grp * GRP:(grp + 1) * GRP, :], in_=vnat)

    def stage2(qi):
        qsl = slice(qi * P, (qi + 1) * P)
        parts = small.tile([P, NKC], F32)
        etiles = []
        for kc in range(NKC):
            ps = psS.tile([P, KC], F32, tag="ps")
            for sub in range(KC // 512):
                nsl = slice(sub * 512, (sub + 1) * 512)
                ksl = slice(kc * KC + sub * 512, kc * KC + (sub + 1) * 512)
                nc.tensor.matmul(ps[:, nsl], QtRe[:, qsl], KtRe[:, ksl],
                                 start=True, stop=False)
                nc.tensor.matmul(ps[:, nsl], QtIm[:, qsl], KtIm[:, ksl],
                                 start=False, stop=True)
            et = esb.tile([P, KC], F32)
            nc.scalar.activation(et, ps, EXP, scale=INV_SCALE,
                                 accum_out=parts[:, kc:kc + 1])
            etiles.append(et)
        rs = small.tile([P, 1], F32)
        nc.vector.reduce_sum(rs, parts, axis=AX)
        nc.vector.reciprocal(recips[:, qi:qi + 1], rs)
        for kc in range(NKC):
            w = wsb.tile([P, KC], F32)
            eng = nc.vector if kc % 2 == 0 else nc.gpsimd
            eng.tensor_scalar_mul(w, etiles[kc], recips[:, qi:qi + 1])
            nc.sync.dma_start(out=Wd[qsl, kc * KC:(kc + 1) * KC], in_=w)

    def stage3_kt(qc, kt, oRe, oIm):
        ksl = slice(kt * P, (kt + 1) * P)
        ps = psS.tile([P, QC], F32, tag="ps")
        for sub in range(QC // 512):
            nsl = slice(sub * 512, (sub + 1) * 512)
            qssl = slice(qc * QC + sub * 512, qc * QC + (sub + 1) * 512)
            nc.tensor.matmul(ps[:, nsl], KtRe[:, ksl], QtRe[:, qssl],
                             start=True, stop=False)
            nc.tensor.matmul(ps[:, nsl], KtIm[:, ksl], QtIm[:, qssl],
                             start=False, stop=True)
        et = etp.tile([P, QC], F32R)
        nc.scalar.activation(et, ps, EXP, scale=INV_SCALE)
        first, last = kt == 0, kt == NK - 1
        for sub in range(QC // 512):
            nsl = slice(sub * 512, (sub + 1) * 512)
            nc.tensor.matmul(oRe[:, nsl], Vsb[:, kt, 0:HD], et[:, nsl],
                             start=first, stop=last)
            nc.tensor.matmul(oIm[:, nsl], Vsb[:, kt, HD:H], et[:, nsl],
                             start=first, stop=last)

    def stage3_epilogue(qc, oRe, oIm):
        oReS = outp.tile([P, QC], F32, tag="ocopy")
        oImS = outp.tile([P, QC], F32, tag="ocopy")
        nc.vector.tensor_copy(out=oReS, in_=oRe)
        nc.vector.tensor_copy(out=oImS, in_=oIm)
        for t in range(QC // P):
            qi = (qc * QC) // P + t
            otile = outp.tile([P, H], F32, tag="ofinal")
            for h, src in ((0, oReS), (1, oImS)):
                pt = psS.tile([P, P], F32, tag="ps")
                nc.tensor.transpose(pt, src[:, t * P:(t + 1) * P], ident)
                nc.vector.tensor_scalar_mul(otile[:, h * HD:(h + 1) * HD],
                                            pt, recips[:, qi:qi + 1])
            nc.sync.dma_start(out=OUTd[qi * P:(qi + 1) * P, :], in_=otile)

    # fine interleave: per q half, alternate one stage2 q-tile with four
    # stage3 k-iterations so PE (stage3-heavy) and ACT/DMA (stage2-heavy)
    # stay concurrently busy.
    QI_PER_QC = NQ // NQC
    KT_PER_QI = NK // QI_PER_QC
    for qc in range(NQC):
        oRe = psOut.tile([P, QC], F32, tag="o")
        oIm = psOut.tile([P, QC], F32, tag="o")
        for i in range(QI_PER_QC):
            stage2(qc * QI_PER_QC + i)
            for kt in range(i * KT_PER_QI, (i + 1) * KT_PER_QI):
                stage3_kt(qc, kt, oRe, oIm)
        stage3_epilogue(qc, oRe, oIm)


def _build():
    nc = bacc.Bacc("TRN2", target_bir_lowering=False)
    Qd = nc.declare_dram_parameter("Q", [QS, H], F32, isOutput=False)
    Kd = nc.declare_dram_parameter("K", [S, H], F32, isOutput=False)
    Vd = nc.declare_dram_parameter("V", [S, H], F32, isOutput=False)
    OUTd = nc.declare_dram_parameter("OUT", [QS, H], F32, isOutput=True)
    Wd = nc.declare_dram_parameter("W", [QS, S], F32, isOutput=True)
    with tile.TileContext(nc) as tc:
        with ExitStack() as ctx:
            _body(ctx, tc, Qd[:], Kd[:], Vd[:], OUTd[:], Wd[:])
    nc.compile()
    return nc


def kernel(Q, K, V):
    global LAST_RESULTS
    Q = np.asarray(Q, dtype=np.float32)
    K = np.asarray(K, dtype=np.float32)
    V = np.asarray(V, dtype=np.float32)

    nc = _build()
    in_maps = []
    for c in range(N_CORES):
        b, sh = c // 2, c % 2
        in_maps.append({
            "Q": np.ascontiguousarray(Q[b, sh * QS:(sh + 1) * QS, :]),
            "K": np.ascontiguousarray(K[b]),
            "V": np.ascontiguousarray(V[b]),
        })
    res = run_bass_kernel_spmd(nc, in_maps, list(range(N_CORES)))
    LAST_RESULTS = res

    out = np.empty((B, S, H), dtype=np.float32)
    weights = np.empty((B, S, S), dtype=np.float32)
    for c in range(N_CORES):
        b, sh = c // 2, c % 2
        out[b, sh * QS:(sh + 1) * QS, :] = res.results[c]["OUT"]
        weights[b, sh * QS:(sh + 1) * QS, :] = res.results[c]["W"]
    return out, weights


# revision 14
# speedup vs baseline: 2.3766x; 2.3766x over previous
"""Complex attention kernel for Trainium2, 8 NeuronCores.

reference math (B=4, S=4096, H=256, h=128):
    scores  = (Qre@Kre^T + Qim@Kim^T) / 16          [B,S,S]
    weights = softmax(scores, -1)                    [B,S,S]  (output 2)
    out     = concat(weights@Vre, weights@Vim, -1)   [B,S,H]  (output 1)

Sharding: 8 shards = (batch b, half of Q sequence). Each core gets
Q[b, s0:s0+2048], K[b], V[b]; no collectives needed.

Per-core pipeline:
  prologue: load K,Q natural -> PE-transpose to [d,seq] f32r layout (fp32
  DMA transpose is unsupported); V loads natural + DVE cast to f32r.
  stage 2 (natural layout, per 128-row q tile):
      S tile [128,1024] = two accumulating f32r matmuls per 512 cols
      E = exp(S/16) on ACT with accum_out rowsum partials
      recip = 1/rowsum (DVE), W = E*recip (DVE/gpsimd alternating), DMA out.
  stage 3 (transposed layout, per 1024-col q chunk):
      S^T tile [k=128, q=1024] recomputed with swapped matmul operands
      E^T = exp(S^T/16); PV accumulates out^T[d, q] over 32 k tiles with V
      natural as stationary; epilogue PE-transposes out^T back to [q, d],
      DVE-scales by the stage-2 reciprocal, DMAs out.
softmax uses no max-subtraction: scores are O(+-7), exp is safe in f32 and
the result is mathematically identical to the max-subtracted reference.
"""

import numpy as np
from contextlib import ExitStack

import concourse.bass as bass
import concourse.bacc as bacc
import concourse.mybir as mybir
import concourse.tile as tile
from concourse.bass_utils import run_bass_kernel_spmd
from concourse.masks import make_identity

B, S, H = 4, 4096, 256
HD = H // 2                      # 128, half hidden (re/im)
N_CORES = 8
QS = B * S // N_CORES            # 2048 q rows per core
INV_SCALE = 1.0 / 16.0           # 1/sqrt(H//2)

F32 = mybir.dt.float32
F32R = mybir.dt.float32r
EXP = mybir.ActivationFunctionType.Exp
AX = mybir.AxisListType.X

P = 128
NQ = QS // P                     # 16 q tiles
NK = S // P                      # 32 k tiles
KC = 1024                        # stage2 k-chunk width (psum/exp tile)
NKC = S // KC                    # 4
QC = 512                         # stage3 q-chunk width
NQC = QS // QC                   # 4

LAST_RESULTS = None              # test harness peeks at exec_time_ns


def _make_pools(ctx, tc):
    # PSUM: tag "ps" 3 slots x 2 banks (S / S^T pairs / all transposes),
    #       tag "o" 2 slots x 1 bank (out^T re+im accumulators) = 8 banks.
    return {
        "singles": ctx.enter_context(tc.tile_pool(name="singles", bufs=1)),
        "psS": ctx.enter_context(tc.tile_pool(name="psS", bufs=3, space="PSUM")),
        "psOut": ctx.enter_context(tc.tile_pool(name="psOut", bufs=2, space="PSUM")),
        "nat": ctx.enter_context(tc.tile_pool(name="nat", bufs=2)),
        "esb": ctx.enter_context(tc.tile_pool(name="esb", bufs=8)),
        "wsb": ctx.enter_context(tc.tile_pool(name="wsb", bufs=4)),
        "etp": ctx.enter_context(tc.tile_pool(name="etp", bufs=6)),
        "outp": ctx.enter_context(tc.tile_pool(name="outp", bufs=4)),
        "small": ctx.enter_context(tc.tile_pool(name="small", bufs=4)),
    }


def _body(ctx, tc, Qd, Kd, Vd, OUTd, Wd, pools=None):
    nc = tc.nc
    if pools is None:
        pools = _make_pools(ctx, tc)
    singles = pools["singles"]
    psS = pools["psS"]
    psOut = pools["psOut"]
    nat = pools["nat"]
    esb = pools["esb"]
    wsb = pools["wsb"]
    etp = pools["etp"]
    outp = pools["outp"]
    small = pools["small"]

    ident = singles.tile([P, P], F32)
    make_identity(nc, ident)

    KtRe = singles.tile([P, S], F32R)
    KtIm = singles.tile([P, S], F32R)
    QtRe = singles.tile([P, QS], F32R)
    QtIm = singles.tile([P, QS], F32R)
    Vsb = singles.tile([P, NK, H], F32R)
    recips = singles.tile([P, NQ], F32)

    # prologue: batched natural loads (8 seq-tiles per DMA), PE transposes
    # with PSUM->SBUF copies split between DVE and ACT. K first (it gates
    # stage2), then Q, then V (cast to f32r on gpsimd).
    GRP = 8

    def load_transpose(src_dram, grp, n_in_grp, dstRe, dstIm):
        nat_t = nat.tile([P, GRP, H], F32, tag="nat")
        src = src_dram.rearrange("(n p) h -> p n h", p=P)
        nc.sync.dma_start(
            out=nat_t[:, :n_in_grp, :],
            in_=src[:, grp * GRP:grp * GRP + n_in_grp, :])
        for j in range(n_in_grp):
            idx = grp * GRP + j
            for h, dst in ((0, dstRe), (1, dstIm)):
                pt = psS.tile([P, P], F32, tag="ps")
                nc.tensor.transpose(pt, nat_t[:, j, h * HD:(h + 1) * HD], ident)
                if (idx + h) % 2 == 0:
                    nc.vector.tensor_copy(
                        out=dst[:, idx * P:(idx + 1) * P], in_=pt)
                else:
                    nc.scalar.copy(out=dst[:, idx * P:(idx + 1) * P], in_=pt)

    # Q group 0 and V first: stage3's first k-pairs depend only on them plus
    # the first K tiles, so PE gets matmul work before all of K lands.
    load_transpose(Qd, 0, GRP, QtRe, QtIm)
    Vre = Vd.rearrange("(n p) h -> p n h", p=P)
    for grp in range(NK // GRP):
        vnat = nat.tile([P, GRP, H], F32, tag="nat")
        nc.sync.dma_start(out=vnat, in_=Vre[:, grp * GRP:(grp + 1) * GRP, :])
        nc.gpsimd.tensor_copy(
            out=Vsb[:, grp * GRP:(grp + 1) * GRP, :], in_=vnat)
    for grp in range(NK // GRP):
        load_transpose(Kd, grp, GRP, KtRe, KtIm)
    for grp in range(1, NQ // GRP):
        load_transpose(Qd, grp, GRP, QtRe, QtIm)

    def stage2(qi):
        qsl = slice(qi * P, (qi + 1) * P)
        parts = small.tile([P, NKC], F32)
        etiles = []
        for kc in range(NKC):
            ps = psS.tile([P, KC], F32, tag="ps")
            for sub in range(KC // 512):
                nsl = slice(sub * 512, (sub + 1) * 512)
                ksl = slice(kc * KC + sub * 512, kc * KC + (sub + 1) * 512)
                nc.tensor.matmul(ps[:, nsl], QtRe[:, qsl], KtRe[:, ksl],
                                 start=True, stop=False)
                nc.tensor.matmul(ps[:, nsl], QtIm[:, qsl], KtIm[:, ksl],
                                 start=False, stop=True)
            et = esb.tile([P, KC], F32)
            nc.scalar.activation(et, ps, EXP, scale=INV_SCALE,
                                 accum_out=parts[:, kc:kc + 1])
            etiles.append(et)
        rs = small.tile([P, 1], F32)
        nc.vector.reduce_sum(rs, parts, axis=AX)
        nc.vector.reciprocal(recips[:, qi:qi + 1], rs)
        for kc in range(NKC):
            w = wsb.tile([P, KC], F32)
            eng = nc.vector if kc % 2 == 0 else nc.gpsimd
            eng.tensor_scalar_mul(w, etiles[kc], recips[:, qi:qi + 1])
            nc.sync.dma_start(out=Wd[qsl, kc * KC:(kc + 1) * KC], in_=w)

    def stage3_ktpair(qc, kt0, oRe, oIm):
        # two k tiles share one [128, 1024] psum tile so the exp stays wide
        qssl = slice(qc * QC, (qc + 1) * QC)
        ps = psS.tile([P, 2 * QC], F32, tag="ps")
        for j, kt in enumerate((kt0, kt0 + 1)):
            ksl = slice(kt * P, (kt + 1) * P)
            nsl = slice(j * QC, (j + 1) * QC)
            nc.tensor.matmul(ps[:, nsl], KtRe[:, ksl], QtRe[:, qssl],
                             start=True, stop=False)
            nc.tensor.matmul(ps[:, nsl], KtIm[:, ksl], QtIm[:, qssl],
                             start=False, stop=True)
        et = etp.tile([P, 2 * QC], F32R)
        nc.scalar.activation(et, ps, EXP, scale=INV_SCALE)
        for j, kt in enumerate((kt0, kt0 + 1)):
            nsl = slice(j * QC, (j + 1) * QC)
            first, last = kt == 0, kt == NK - 1
            nc.tensor.matmul(oRe, Vsb[:, kt, 0:HD], et[:, nsl],
                             start=first, stop=last)
            nc.tensor.matmul(oIm, Vsb[:, kt, HD:H], et[:, nsl],
                             start=first, stop=last)

    def stage3_epilogue(qc, oRe, oIm):
        oReS = outp.tile([P, QC], F32, tag="ocopy")
        oImS = outp.tile([P, QC], F32, tag="ocopy")
        nc.vector.tensor_copy(out=oReS, in_=oRe)
        nc.vector.tensor_copy(out=oImS, in_=oIm)
        for t in range(QC // P):
            qi = (qc * QC) // P + t
            otile = outp.tile([P, H], F32, tag="ofinal")
            for h, src in ((0, oReS), (1, oImS)):
                pt = psS.tile([P, P], F32, tag="ps")
                nc.tensor.transpose(pt, src[:, t * P:(t + 1) * P], ident)
                nc.vector.tensor_scalar_mul(otile[:, h * HD:(h + 1) * HD],
                                            pt, recips[:, qi:qi + 1])
            nc.sync.dma_start(out=OUTd[qi * P:(qi + 1) * P, :], in_=otile)

    # fine interleave: per q chunk, alternate one stage2 q-tile with eight
    # stage3 k-pairs so PE (stage3-heavy) and ACT/DMA (stage2-heavy) stay
    # concurrently busy.
    QI_PER_QC = NQ // NQC            # 4
    PAIRS_PER_QI = (NK // 2) // QI_PER_QC  # 4
    for qc in range(NQC):
        oRe = psOut.tile([P, QC], F32, tag="o")
        oIm = psOut.tile([P, QC], F32, tag="o")
        for i in range(QI_PER_QC):
            stage2(qc * QI_PER_QC + i)
            for pr in range(i * PAIRS_PER_QI, (i + 1) * PAIRS_PER_QI):
                stage3_ktpair(qc, 2 * pr, oRe, oIm)
        stage3_epilogue(qc, oRe, oIm)


def _build():
    nc = bacc.Bacc("TRN2", target_bir_lowering=False)
    Qd = nc.declare_dram_parameter("Q", [QS, H], F32, isOutput=False)
    Kd = nc.declare_dram_parameter("K", [S, H], F32, isOutput=False)
    Vd = nc.declare_dram_parameter("V", [S, H], F32, isOutput=False)
    OUTd = nc.declare_dram_parameter("OUT", [QS, H], F32, isOutput=True)
    Wd = nc.declare_dram_parameter("W", [QS, S], F32, isOutput=True)
    with tile.TileContext(nc) as tc:
        with ExitStack() as ctx:
            _body(ctx, tc, Qd[:], Kd[:], Vd[:], OUTd[:], Wd[:])
    nc.compile()
    return nc


def kernel(Q, K, V):
    global LAST_RESULTS
    Q = np.asarray(Q, dtype=np.float32)
    K = np.asarray(K, dtype=np.float32)
    V = np.asarray(V, dtype=np.float32)

    nc = _build()
    in_maps = []
    for c in range(N_CORES):
        b, sh = c // 2, c % 2
        in_maps.append({
            "Q": np.ascontiguousarray(Q[b, sh * QS:(sh + 1) * QS, :]),
            "K": np.ascontiguousarray(K[b]),
            "V": np.ascontiguousarray(V[b]),
        })
    res = run_bass_kernel_spmd(nc, in_maps, list(range(N_CORES)))
    LAST_RESULTS = res

    out = np.empty((B, S, H), dtype=np.float32)
    weights = np.empty((B, S, S), dtype=np.float32)
    for c in range(N_CORES):
        b, sh = c // 2, c % 2
        out[b, sh * QS:(sh + 1) * QS, :] = res.results[c]["OUT"]
        weights[b, sh * QS:(sh + 1) * QS, :] = res.results[c]["W"]
    return out, weights
